# revision 28
# baseline (speedup 1.0000x reference)
"""Bahdanau (additive) attention kernel for Trainium2, 8 NeuronCores.

Problem shapes: B=256, S=512, E=DEC=512, U=64.
  dec  = dhs @ w1 + b1                      [B, 1, U]
  enc  = eo @ w2 + b2                       [B, S, U]
  score= tanh(dec + enc) @ v + vb           [B, S, 1]
  att  = softmax(score, axis=1)             [B, S, 1]
  ctx  = sum_s att * eo                     [B, E]

Strategy (data-parallel over B, 32 batches/core):
  - The PE contracts over the partition dim only, so the enc-projection
    (contract over E) needs eo with E on partitions while the context
    (contract over S) needs S on partitions. fp32 on-chip transposition is
    ~2x slower than the whole memory roofline, so the host ships TWO bf16
    copies of eo (natural [b,s,e] and transposed [b,e,s]) - same total bytes
    as fp32-once, so the kernel still runs at the 32MB/core HBM roofline.
  - Host pre-shuffles both copies into a group-major [g][p][b][k][x] layout
    (row = 128k+p) so each group-DMA reads one contiguous 16KB run per
    partition (maximal descriptors, ~410-420GB/s per ring when busy);
    batches load in tapered groups (1,1,2,4,4,...) split across both HWDGE
    rings (natural on sync, transposed on scalar) with triple buffering.
  - All accumulation is fp32 in PSUM; softmax normalization is deferred
    (ctx_unnorm scaled by 1/sum at the end) so no cross-partition broadcast
    of scalars is ever needed.
  - v_bias shifts every score equally -> softmax-invariant -> dropped.
"""

import numpy as np
import ml_dtypes

B, S, E, U = 256, 512, 512, 64
NCORES = 8
BL = B // NCORES  # 32 batches per core
KC = 4            # 512 = 4 chunks of 128 partitions
GB = 4            # batches per DMA group

_CACHE = {}


def _group_sizes(bl):
    # tapered: small leading groups so compute starts ~2us in; 4-batch (2MB,
    # 16KB-descriptor) groups for the bulk of the stream
    if bl >= 8:
        return [1, 1, 2] + [GB] * ((bl - 4) // GB)
    return [1] * bl


def _build(bl=BL):
    from contextlib import ExitStack
    from concourse import bacc, tile, mybir

    f32 = mybir.dt.float32
    bf16 = mybir.dt.bfloat16
    AF = mybir.ActivationFunctionType

    nc = bacc.Bacc(
        "TRN2",
        target_bir_lowering=False,
        debug=False,
        enable_asserts=True,
        num_devices=NCORES,
    )

    # host-shuffled GROUP-MAJOR layouts: flat [sum_g 128*gsz*KC*X] laid out as
    # [g][p][b][k][x] (row = 128*k + p), so one group-load's per-partition data
    # is a single contiguous gsz*4KB DRAM run -> up-to-16KB DMA descriptors
    eo_nat = nc.dram_tensor("eo_nat", [bl * 128 * KC * E], bf16,
                            kind="ExternalInput").ap()
    eo_t = nc.dram_tensor("eo_t", [bl * 128 * KC * S], bf16,
                          kind="ExternalInput").ap()
    dhsT = nc.dram_tensor("dhsT", [E, bl], f32, kind="ExternalInput").ap()
    w1 = nc.dram_tensor("w1", [E, U], f32, kind="ExternalInput").ap()
    w2 = nc.dram_tensor("w2", [E, U], bf16, kind="ExternalInput").ap()
    wb = nc.dram_tensor("wb", [U, 1], f32, kind="ExternalInput").ap()
    vv = nc.dram_tensor("vv", [U, 1], bf16, kind="ExternalInput").ap()
    att_o = nc.dram_tensor("att_o", [bl, S], f32, kind="ExternalOutput").ap()
    ctx_o = nc.dram_tensor("ctx_o", [bl, S], f32, kind="ExternalOutput").ap()

    with tile.TileContext(nc) as tc, ExitStack() as ctx:
        singles = ctx.enter_context(tc.tile_pool(name="singles", bufs=1))
        natp = ctx.enter_context(tc.tile_pool(name="natp", bufs=4))
        ttp = ctx.enter_context(tc.tile_pool(name="ttp", bufs=4))
        workp = ctx.enter_context(tc.tile_pool(name="workp", bufs=4))
        smol = ctx.enter_context(tc.tile_pool(name="smol", bufs=6))
        stagep = ctx.enter_context(tc.tile_pool(name="stagep", bufs=2))
        gps = ctx.enter_context(tc.tile_pool(name="gps", bufs=2, space="PSUM"))
        srp = ctx.enter_context(tc.tile_pool(name="srp", bufs=2, space="PSUM"))
        scp = ctx.enter_context(tc.tile_pool(name="scp", bufs=2, space="PSUM"))
        cxp = ctx.enter_context(tc.tile_pool(name="cxp", bufs=2, space="PSUM"))

        # ---- weights / small inputs ----
        w1_sb = singles.tile([128, KC, U], f32)
        nc.sync.dma_start(w1_sb, w1.rearrange("(k p) u -> p k u", p=128))
        dhsT_sb = singles.tile([128, KC, bl], f32)
        nc.sync.dma_start(dhsT_sb, dhsT.rearrange("(k p) b -> p k b", p=128))
        w2_sb = singles.tile([128, KC, U], bf16)
        nc.sync.dma_start(w2_sb, w2.rearrange("(k p) u -> p k u", p=128))
        wb_sb = singles.tile([U, 1], f32)
        nc.sync.dma_start(wb_sb, wb)
        v_sb = singles.tile([U, 1], bf16)
        nc.sync.dma_start(v_sb, vv)

        # ---- decT[u, b] = w1.T @ dhs.T + (w1_bias + w2_bias) ----
        dec_ps = gps.tile([U, bl], f32, tag="g")
        for k in range(KC):
            nc.tensor.matmul(
                dec_ps, w1_sb[:, k, :], dhsT_sb[:, k, :],
                start=(k == 0), stop=(k == KC - 1),
            )
        decT_sb = singles.tile([U, bl], f32)
        nc.scalar.activation(decT_sb, dec_ps, AF.Identity, bias=wb_sb[:, 0:1])

        sizes = _group_sizes(bl)
        assert sum(sizes) == bl
        starts = [sum(sizes[:g]) for g in range(len(sizes))]
        ngroups = len(sizes)

        def load_group(g):
            # natural stream on the sync HWDGE ring, transposed stream on the
            # scalar ring - two independent FIFOs, no head-of-line blocking
            g0, gsz = starts[g], sizes[g]
            off = g0 * 128 * KC * E
            n = 128 * gsz * KC * E
            nat_t = natp.tile([128, gsz, KC, E], bf16, name="nat_t", tag="nat_t")
            nc.sync.dma_start(nat_t, eo_nat[off:off + n].rearrange(
                "(p b k e) -> p b k e", p=128, b=gsz, k=KC))
            tt_t = ttp.tile([128, gsz, KC, S], bf16, name="tt_t", tag="tt_t")
            nc.scalar.dma_start(tt_t, eo_t[off:off + n].rearrange(
                "(p b k s) -> p b k s", p=128, b=gsz, k=KC))
            return nat_t, tt_t

        def proj_b(tt_t, i):
            # G[u, s] = w2.T @ eo[b].T  (contract E on partitions)
            g_ps = gps.tile([U, S], f32, tag="g", name="g_ps")
            for k in range(KC):
                nc.tensor.matmul(
                    g_ps, w2_sb[:, k, :], tt_t[:, i, k, :],
                    start=(k == 0), stop=(k == KC - 1),
                )
            return g_ps

        def tanh_b(g_ps, b):
            # T[u, s] = tanh(G + dec[:, b] + biases)
            t_sb = workp.tile([U, S], bf16, name="t_sb", tag="t_sb")
            nc.scalar.activation(t_sb, g_ps, AF.Tanh, bias=decT_sb[:, b:b + 1])
            return t_sb

        # software pipeline prologue: group g and g+1 loading; T(0) ready
        groups = [load_group(0), load_group(1)]
        t_cur = tanh_b(proj_b(groups[0][1], 0), 0)

        b = 0
        for g in range(ngroups):
            nat_t, tt_t = groups[g]
            gsz = sizes[g]
            att_st = stagep.tile([1, gsz * S], f32, name="att_st", tag="att_st")
            ctx_st = stagep.tile([1, gsz * S], f32, name="ctx_st", tag="ctx_st")

            for i in range(gsz):
                t_sb = t_cur

                # score row [1, s] = v.T @ T ; score col chunks = T.T @ v
                # (emitted FIRST on the PE so ACT's Exp can start while the
                #  PE works on the next batch's enc-projection)
                sr_ps = srp.tile([1, S], f32, name="sr_ps", tag="sr")
                nc.tensor.matmul(sr_ps, v_sb, t_sb, start=True, stop=True)
                sc_ps = scp.tile([128, KC], f32, name="sc_ps", tag="sc")
                for j in range(KC):
                    nc.tensor.matmul(
                        sc_ps[:, j:j + 1], t_sb[:, j * 128:(j + 1) * 128], v_sb,
                        start=True, stop=True,
                    )
                ec_sb = smol.tile([128, KC], bf16, name="ec_sb")
                nc.scalar.activation(ec_sb, sc_ps, AF.Exp)

                # next batch's enc-proj fills the PE while ACT runs
                g_nxt = None
                if i + 1 < gsz:
                    g_nxt = proj_b(tt_t, i + 1)
                elif g + 1 < ngroups:
                    g_nxt = proj_b(groups[g + 1][1], 0)

                # ctx_unnorm [1, e] = sum_s exp(score[s]) * eo[b, s, :]
                cx_ps = cxp.tile([1, E], f32, name="cx_ps", tag="cx")
                for j in range(KC):
                    nc.tensor.matmul(
                        cx_ps, ec_sb[:, j:j + 1], nat_t[:, i, j, :],
                        start=(j == 0), stop=(j == KC - 1),
                    )

                # exp row + sum -> 1/sum (off the PE critical path)
                er_sb = smol.tile([1, S], f32, name="er_sb")
                sum_sb = smol.tile([1, 1], f32, name="sum_sb")
                nc.scalar.activation(er_sb, sr_ps, AF.Exp, accum_out=sum_sb)

                # next batch's tanh right after this batch's ACT tail
                if g_nxt is not None:
                    t_cur = tanh_b(g_nxt, b + 1)

                inv_sb = smol.tile([1, 1], f32, name="inv_sb")
                nc.vector.reciprocal(inv_sb, sum_sb)

                # normalize + stage outputs
                nc.vector.tensor_scalar_mul(
                    att_st[:, i * S:(i + 1) * S], er_sb, inv_sb)
                nc.vector.tensor_scalar_mul(
                    ctx_st[:, i * S:(i + 1) * S], cx_ps, inv_sb)
                b += 1

            # group output stores on the (otherwise idle) SWDGE ring
            g0 = starts[g]
            nc.gpsimd.dma_start(
                att_o[g0:g0 + gsz].rearrange("b s -> (b s)")
                .rearrange("(o n) -> o n", o=1), att_st)
            nc.gpsimd.dma_start(
                ctx_o[g0:g0 + gsz].rearrange("b s -> (b s)")
                .rearrange("(o n) -> o n", o=1), ctx_st)

            if g + 2 < ngroups:
                groups.append(load_group(g + 2))

    nc.compile()
    return nc


def _get_nc():
    if "nc" not in _CACHE:
        _CACHE["nc"] = _build()
    return _CACHE["nc"]


def _prep_inputs(decoder_hidden_state, encoder_output, w1_kernel, w1_bias,
                 w2_kernel, w2_bias, v_kernel, v_bias):
    bf = ml_dtypes.bfloat16
    eo = np.ascontiguousarray(np.asarray(encoder_output, dtype=np.float32))
    dhs = np.asarray(decoder_hidden_state, dtype=np.float32)
    w1 = np.ascontiguousarray(np.asarray(w1_kernel, dtype=np.float32))
    w2 = np.asarray(w2_kernel, dtype=np.float32).astype(bf)
    wb = (np.asarray(w1_bias, dtype=np.float32)
          + np.asarray(w2_bias, dtype=np.float32)).reshape(U, 1)
    wb = np.ascontiguousarray(wb)
    v16 = np.ascontiguousarray(np.asarray(v_kernel, dtype=np.float32).astype(bf))

    eo16 = eo.astype(bf)  # [B, S, E]
    # shuffled layouts (see _build): [b, p, k, x], row = 128*k + p
    nat_all = eo16.reshape(B, KC, 128, E).transpose(0, 2, 1, 3)
    tt_all = eo16.transpose(0, 2, 1).reshape(B, KC, 128, S).transpose(0, 2, 1, 3)

    sizes = _group_sizes(BL)
    starts = [sum(sizes[:g]) for g in range(len(sizes))]

    def group_major(arr):  # [BL, 128, KC, X] -> flat [g][p][b][k][x]
        return np.concatenate([
            np.ascontiguousarray(
                arr[s0:s0 + gsz].transpose(1, 0, 2, 3)).reshape(-1)
            for s0, gsz in zip(starts, sizes)])

    in_maps = []
    for c in range(NCORES):
        sl = slice(c * BL, (c + 1) * BL)
        dT = np.ascontiguousarray(dhs[sl].T)
        in_maps.append({
            "eo_nat": group_major(nat_all[sl]),
            "eo_t": group_major(tt_all[sl]),
            "dhsT": dT,
            "w1": w1,
            "w2": np.ascontiguousarray(w2),
            "wb": wb,
            "vv": v16,
        })
    return in_maps


def kernel(decoder_hidden_state, encoder_output, w1_kernel, w1_bias,
           w2_kernel, w2_bias, v_kernel, v_bias, _trace=False, _tmpdir=None):
    from concourse.bass_utils import run_bass_kernel_spmd

    nc = _get_nc()
    in_maps = _prep_inputs(decoder_hidden_state, encoder_output, w1_kernel,
                           w1_bias, w2_kernel, w2_bias, v_kernel, v_bias)
    res = run_bass_kernel_spmd(
        nc, in_maps, core_ids=list(range(NCORES)),
        trace=_trace, tmpdir=_tmpdir,
    )
    _CACHE["last_result"] = res

    ctx = np.concatenate([r["ctx_o"] for r in res.results], axis=0)
    att = np.concatenate([r["att_o"] for r in res.results], axis=0)
    return (ctx.astype(np.float32), att.astype(np.float32).reshape(B, S, 1))


# revision 29
# speedup vs baseline: 1.1516x; 1.1516x over previous
"""Bahdanau (additive) attention kernel for Trainium2, 8 NeuronCores.

Problem shapes: B=256, S=512, E=DEC=512, U=64.
  dec  = dhs @ w1 + b1                      [B, 1, U]
  enc  = eo @ w2 + b2                       [B, S, U]
  score= tanh(dec + enc) @ v + vb           [B, S, 1]
  att  = softmax(score, axis=1)             [B, S, 1]
  ctx  = sum_s att * eo                     [B, E]

Strategy (data-parallel over B, 32 batches/core):
  - The PE contracts over the partition dim only, so the enc-projection
    (contract over E) needs eo with E on partitions while the context
    (contract over S) needs S on partitions. fp32 on-chip transposition is
    ~2x slower than the whole memory roofline, so the host ships TWO bf16
    copies of eo (natural [b,s,e] and transposed [b,e,s]) - same total bytes
    as fp32-once, so the kernel still runs at the 32MB/core HBM roofline.
  - Host pre-shuffles both copies into a group-major [g][p][b][k][x] layout
    (row = 128k+p) so each group-DMA reads one contiguous 16KB run per
    partition (maximal descriptors, ~410-420GB/s per ring when busy);
    batches load in tapered groups (1,1,2,4,4,...) split across both HWDGE
    rings (natural on sync, transposed on scalar) with triple buffering.
  - All accumulation is fp32 in PSUM; softmax normalization is deferred
    (ctx_unnorm scaled by 1/sum at the end) so no cross-partition broadcast
    of scalars is ever needed.
  - v_bias shifts every score equally -> softmax-invariant -> dropped.
"""

import numpy as np
import ml_dtypes

B, S, E, U = 256, 512, 512, 64
NCORES = 8
BL = B // NCORES  # 32 batches per core
KC = 4            # 512 = 4 chunks of 128 partitions
GB = 4            # batches per DMA group

_CACHE = {}


def _group_sizes(bl):
    # tapered: small leading groups so compute starts ~2us in; 4-batch (2MB,
    # 16KB-descriptor) groups for the bulk of the stream
    if bl >= 8:
        return [1, 1, 2] + [GB] * ((bl - 4) // GB)
    return [1] * bl


def _build(bl=BL):
    from contextlib import ExitStack
    from concourse import bacc, tile, mybir

    f32 = mybir.dt.float32
    bf16 = mybir.dt.bfloat16
    AF = mybir.ActivationFunctionType

    nc = bacc.Bacc(
        "TRN2",
        target_bir_lowering=False,
        debug=False,
        enable_asserts=True,
        num_devices=NCORES,
    )

    # host-shuffled GROUP-MAJOR layouts: flat [sum_g 128*gsz*KC*X] laid out as
    # [g][p][b][k][x] (row = 128*k + p), so one group-load's per-partition data
    # is a single contiguous gsz*4KB DRAM run -> up-to-16KB DMA descriptors
    eo_nat = nc.dram_tensor("eo_nat", [bl * 128 * KC * E], bf16,
                            kind="ExternalInput").ap()
    eo_t = nc.dram_tensor("eo_t", [bl * 128 * KC * S], bf16,
                          kind="ExternalInput").ap()
    dhsT = nc.dram_tensor("dhsT", [E, bl], f32, kind="ExternalInput").ap()
    w1 = nc.dram_tensor("w1", [E, U], f32, kind="ExternalInput").ap()
    w2 = nc.dram_tensor("w2", [E, U], bf16, kind="ExternalInput").ap()
    wb = nc.dram_tensor("wb", [U, 1], f32, kind="ExternalInput").ap()
    vv = nc.dram_tensor("vv", [U, 1], bf16, kind="ExternalInput").ap()
    att_o = nc.dram_tensor("att_o", [bl, S], f32, kind="ExternalOutput").ap()
    ctx_o = nc.dram_tensor("ctx_o", [bl, S], f32, kind="ExternalOutput").ap()

    with tile.TileContext(nc) as tc, ExitStack() as ctx:
        singles = ctx.enter_context(tc.tile_pool(name="singles", bufs=1))
        natp = ctx.enter_context(tc.tile_pool(name="natp", bufs=3))
        ttp = ctx.enter_context(tc.tile_pool(name="ttp", bufs=3))
        workp = ctx.enter_context(tc.tile_pool(name="workp", bufs=3))
        smol = ctx.enter_context(tc.tile_pool(name="smol", bufs=4))
        stagep = ctx.enter_context(tc.tile_pool(name="stagep", bufs=2))
        gps = ctx.enter_context(tc.tile_pool(name="gps", bufs=2, space="PSUM"))
        srp = ctx.enter_context(tc.tile_pool(name="srp", bufs=2, space="PSUM"))
        scp = ctx.enter_context(tc.tile_pool(name="scp", bufs=2, space="PSUM"))
        cxp = ctx.enter_context(tc.tile_pool(name="cxp", bufs=2, space="PSUM"))

        # ---- weights / small inputs ----
        w1_sb = singles.tile([128, KC, U], f32)
        nc.sync.dma_start(w1_sb, w1.rearrange("(k p) u -> p k u", p=128))
        dhsT_sb = singles.tile([128, KC, bl], f32)
        nc.sync.dma_start(dhsT_sb, dhsT.rearrange("(k p) b -> p k b", p=128))
        w2_sb = singles.tile([128, KC, U], bf16)
        nc.sync.dma_start(w2_sb, w2.rearrange("(k p) u -> p k u", p=128))
        wb_sb = singles.tile([U, 1], f32)
        nc.sync.dma_start(wb_sb, wb)
        v_sb = singles.tile([U, 1], bf16)
        nc.sync.dma_start(v_sb, vv)

        # ---- decT[u, b] = w1.T @ dhs.T + (w1_bias + w2_bias) ----
        dec_ps = gps.tile([U, bl], f32, tag="g")
        for k in range(KC):
            nc.tensor.matmul(
                dec_ps, w1_sb[:, k, :], dhsT_sb[:, k, :],
                start=(k == 0), stop=(k == KC - 1),
            )
        decT_sb = singles.tile([U, bl], f32)
        nc.scalar.activation(decT_sb, dec_ps, AF.Identity, bias=wb_sb[:, 0:1])

        sizes = _group_sizes(bl)
        assert sum(sizes) == bl
        starts = [sum(sizes[:g]) for g in range(len(sizes))]
        ngroups = len(sizes)

        def load_group(g):
            # natural stream on the sync HWDGE ring, transposed stream on the
            # scalar ring - two independent FIFOs, no head-of-line blocking
            g0, gsz = starts[g], sizes[g]
            off = g0 * 128 * KC * E
            n = 128 * gsz * KC * E
            nat_t = natp.tile([128, gsz, KC, E], bf16, name="nat_t", tag="nat_t")
            nc.sync.dma_start(nat_t, eo_nat[off:off + n].rearrange(
                "(p b k e) -> p b k e", p=128, b=gsz, k=KC))
            tt_t = ttp.tile([128, gsz, KC, S], bf16, name="tt_t", tag="tt_t")
            nc.scalar.dma_start(tt_t, eo_t[off:off + n].rearrange(
                "(p b k s) -> p b k s", p=128, b=gsz, k=KC))
            return nat_t, tt_t

        def proj_b(tt_t, i):
            # G[u, s] = w2.T @ eo[b].T  (contract E on partitions)
            g_ps = gps.tile([U, S], f32, tag="g", name="g_ps")
            for k in range(KC):
                nc.tensor.matmul(
                    g_ps, w2_sb[:, k, :], tt_t[:, i, k, :],
                    start=(k == 0), stop=(k == KC - 1),
                )
            return g_ps

        def tanh_b(g_ps, b):
            # T[u, s] = tanh(G + dec[:, b] + biases)
            t_sb = workp.tile([U, S], bf16, name="t_sb", tag="t_sb")
            nc.scalar.activation(t_sb, g_ps, AF.Tanh, bias=decT_sb[:, b:b + 1])
            return t_sb

        # software pipeline prologue: group g and g+1 loading; T(0) ready
        groups = [load_group(0), load_group(1)]
        t_cur = tanh_b(proj_b(groups[0][1], 0), 0)

        b = 0
        for g in range(ngroups):
            nat_t, tt_t = groups[g]
            gsz = sizes[g]
            att_st = stagep.tile([1, gsz * S], f32, name="att_st", tag="att_st")
            ctx_st = stagep.tile([1, gsz * S], f32, name="ctx_st", tag="ctx_st")

            for i in range(gsz):
                t_sb = t_cur

                # score row [1, s] = v.T @ T ; score col chunks = T.T @ v
                # (emitted FIRST on the PE so ACT's Exp can start while the
                #  PE works on the next batch's enc-projection)
                sr_ps = srp.tile([1, S], f32, name="sr_ps", tag="sr")
                nc.tensor.matmul(sr_ps, v_sb, t_sb, start=True, stop=True)
                sc_ps = scp.tile([128, KC], f32, name="sc_ps", tag="sc")
                for j in range(KC):
                    nc.tensor.matmul(
                        sc_ps[:, j:j + 1], t_sb[:, j * 128:(j + 1) * 128], v_sb,
                        start=True, stop=True,
                    )
                ec_sb = smol.tile([128, KC], bf16, name="ec_sb")
                nc.scalar.activation(ec_sb, sc_ps, AF.Exp)

                # next batch's enc-proj fills the PE while ACT runs
                g_nxt = None
                if i + 1 < gsz:
                    g_nxt = proj_b(tt_t, i + 1)
                elif g + 1 < ngroups:
                    g_nxt = proj_b(groups[g + 1][1], 0)

                # ctx_unnorm [1, e] = sum_s exp(score[s]) * eo[b, s, :]
                cx_ps = cxp.tile([1, E], f32, name="cx_ps", tag="cx")
                for j in range(KC):
                    nc.tensor.matmul(
                        cx_ps, ec_sb[:, j:j + 1], nat_t[:, i, j, :],
                        start=(j == 0), stop=(j == KC - 1),
                    )

                # exp row + sum -> 1/sum (off the PE critical path)
                er_sb = smol.tile([1, S], f32, name="er_sb")
                sum_sb = smol.tile([1, 1], f32, name="sum_sb")
                nc.scalar.activation(er_sb, sr_ps, AF.Exp, accum_out=sum_sb)

                # next batch's tanh right after this batch's ACT tail
                if g_nxt is not None:
                    t_cur = tanh_b(g_nxt, b + 1)

                inv_sb = smol.tile([1, 1], f32, name="inv_sb")
                nc.vector.reciprocal(inv_sb, sum_sb)

                # normalize + stage outputs
                nc.vector.tensor_scalar_mul(
                    att_st[:, i * S:(i + 1) * S], er_sb, inv_sb)
                nc.vector.tensor_scalar_mul(
                    ctx_st[:, i * S:(i + 1) * S], cx_ps, inv_sb)
                b += 1

            # group output stores on the (otherwise idle) SWDGE ring
            g0 = starts[g]
            nc.gpsimd.dma_start(
                att_o[g0:g0 + gsz].rearrange("b s -> (b s)")
                .rearrange("(o n) -> o n", o=1), att_st)
            nc.gpsimd.dma_start(
                ctx_o[g0:g0 + gsz].rearrange("b s -> (b s)")
                .rearrange("(o n) -> o n", o=1), ctx_st)

            if g + 2 < ngroups:
                groups.append(load_group(g + 2))

    nc.compile()
    return nc


def _get_nc():
    if "nc" not in _CACHE:
        _CACHE["nc"] = _build()
    return _CACHE["nc"]


def _prep_inputs(decoder_hidden_state, encoder_output, w1_kernel, w1_bias,
                 w2_kernel, w2_bias, v_kernel, v_bias):
    bf = ml_dtypes.bfloat16
    eo = np.ascontiguousarray(np.asarray(encoder_output, dtype=np.float32))
    dhs = np.asarray(decoder_hidden_state, dtype=np.float32)
    w1 = np.ascontiguousarray(np.asarray(w1_kernel, dtype=np.float32))
    w2 = np.asarray(w2_kernel, dtype=np.float32).astype(bf)
    wb = (np.asarray(w1_bias, dtype=np.float32)
          + np.asarray(w2_bias, dtype=np.float32)).reshape(U, 1)
    wb = np.ascontiguousarray(wb)
    v16 = np.ascontiguousarray(np.asarray(v_kernel, dtype=np.float32).astype(bf))

    eo16 = eo.astype(bf)  # [B, S, E]
    # shuffled layouts (see _build): [b, p, k, x], row = 128*k + p
    nat_all = eo16.reshape(B, KC, 128, E).transpose(0, 2, 1, 3)
    tt_all = eo16.transpose(0, 2, 1).reshape(B, KC, 128, S).transpose(0, 2, 1, 3)

    sizes = _group_sizes(BL)
    starts = [sum(sizes[:g]) for g in range(len(sizes))]

    def group_major(arr):  # [BL, 128, KC, X] -> flat [g][p][b][k][x]
        return np.concatenate([
            np.ascontiguousarray(
                arr[s0:s0 + gsz].transpose(1, 0, 2, 3)).reshape(-1)
            for s0, gsz in zip(starts, sizes)])

    in_maps = []
    for c in range(NCORES):
        sl = slice(c * BL, (c + 1) * BL)
        dT = np.ascontiguousarray(dhs[sl].T)
        in_maps.append({
            "eo_nat": group_major(nat_all[sl]),
            "eo_t": group_major(tt_all[sl]),
            "dhsT": dT,
            "w1": w1,
            "w2": np.ascontiguousarray(w2),
            "wb": wb,
            "vv": v16,
        })
    return in_maps


def kernel(decoder_hidden_state, encoder_output, w1_kernel, w1_bias,
           w2_kernel, w2_bias, v_kernel, v_bias, _trace=False, _tmpdir=None):
    from concourse.bass_utils import run_bass_kernel_spmd

    nc = _get_nc()
    in_maps = _prep_inputs(decoder_hidden_state, encoder_output, w1_kernel,
                           w1_bias, w2_kernel, w2_bias, v_kernel, v_bias)
    res = run_bass_kernel_spmd(
        nc, in_maps, core_ids=list(range(NCORES)),
        trace=_trace, tmpdir=_tmpdir,
    )
    _CACHE["last_result"] = res

    ctx = np.concatenate([r["ctx_o"] for r in res.results], axis=0)
    att = np.concatenate([r["att_o"] for r in res.results], axis=0)
    return (ctx.astype(np.float32), att.astype(np.float32).reshape(B, S, 1))
